# revision 6
# baseline (speedup 1.0000x reference)
"""Trainium2 Bass kernel for nn_Attention_8924942041930 (sparse_attention).

Reference computation (per batch of N=384 tokens = [t(64) | it(64) | s(256)]):
  qkv = x @ w_qkv
  mix attention: (t+s) queries over all N keys
  inherent attention: it queries over it keys only
  out = concat(t, it, s) @ w_proj + b_proj

Sharding: data-parallel over batch, 8 batches per NeuronCore (64 total / 8).

Per-core dataflow (all matmuls bf16 with f32 PSUM accumulation):
  - host supplies x^T [768, 3072] bf16 so the f-contraction lands on SBUF
    partitions with no on-chip transposes
  - qk^T = w_qkv^T @ x^T per batch -> [hd, token] per head (head pair per
    128-partition chunk: even head rows 0-63, odd head rows 64-127)
  - v = x @ w_v -> [token, hd] per head (PV lhsT needs token on partitions)
  - S^T[k, q] = k^T.T @ q^T per (head, k-chunk); K=64 so head pairs are
    row-packed via tile_position rows 0/64 into one 2-bank PSUM tile per
    (pair, k-chunk) so a single ScalarE exp covers both heads (768 elems,
    amortizing the ~185ns activation bubble)
  - attn^T = exp(S^T * scale) (logits are tiny: no max-subtract)
  - ts^T = v.T @ attn^T col-packed per head pair (tile_position cols 0/64);
    the inherent block (it x it) rides in the same PSUM tile at cols N:NW
  - denominators = ones.T @ attn^T as M=1 matmuls col-tiled 4 heads/bank
  - normalize: fast reciprocal + DRAM-bounce partition-broadcast + DVE mult
  - out^T = w_proj^T @ attnout^T + b_proj -> bf16 -> host transposes back
    and upconverts to f32

Scheduling: the PE pays a pipeline-drain penalty (~110ns/boundary, measured)
every time the instruction stream switches between tiled (tile_position) and
untiled matmuls.  So tiled attention work is emitted in large clumps
([6 S mms] / [16 sums mms] / [16 PV mms]) with dense GEMM filler (qkv of
batch b+1, proj of batch b-1, in 3-matmul pieces) injected only between
clumps — few boundaries, and the filler also hides ScalarE exp latency and
PSUM recycling.  Unconsumed proj pieces carry across batches (capped) so the
final batch's attention still has dense filler.

PSUM (8 banks): tag "sp" = 3 bufs x 4KB (2-bank tiles: S pair-chunk tiles,
and the sums / PV+inherent tiles ride the same ring) = 6 banks; tag "w" =
2 bufs x 2KB for dense filler chains = 2 banks.
"""

import sys

import numpy as np

if "/opt/trn_rl_repo" not in sys.path:
    sys.path.insert(0, "/opt/trn_rl_repo")

import ml_dtypes

B = 64
N = 384
DIM = 768
H = 12
HD = 64
T_SIZE = 64
S_SIZE = 256
SCALE = HD ** -0.5
NCORES = 8
BPC = B // NCORES  # batches per core
FCH = DIM // 128  # feature chunks of 128
P = 128
NW = N + T_SIZE  # pv/sums/bcast width: mix cols 0:N, inherent cols N:N+64

BF16 = ml_dtypes.bfloat16

PROJ_CARRY_CAP = 24  # max carried proj filler pieces (two batches' worth)


def build(n_batches=BPC, sim_safe=False):
    """Build the per-core Bass program (SPMD: same NEFF, per-core data)."""
    import concourse.mybir as mybir
    import concourse.tile as tile
    from concourse import bacc
    from collections import deque
    from contextlib import ExitStack

    bf16 = mybir.dt.bfloat16
    f32 = mybir.dt.float32
    Exp = mybir.ActivationFunctionType.Exp
    mult = mybir.AluOpType.mult
    ntok = n_batches * N

    nc = bacc.Bacc("TRN2", target_bir_lowering=False, debug=False,
                   num_devices=NCORES)
    xT = nc.dram_tensor("xT", [DIM, ntok], bf16, kind="ExternalInput")
    wqkv = nc.dram_tensor("wqkv", [DIM, 3 * DIM], bf16, kind="ExternalInput")
    wproj = nc.dram_tensor("wproj", [DIM, DIM], bf16, kind="ExternalInput")
    bproj = nc.dram_tensor("bproj", [DIM], f32, kind="ExternalInput")
    outT = nc.dram_tensor("outT", [DIM, ntok], bf16, kind="ExternalOutput")

    xT_r = xT.rearrange("(o p) t -> p o t", p=P)
    wqkv_r = wqkv.rearrange("(o p) c -> p o c", p=P)
    wproj_r = wproj.rearrange("(o p) c -> p o c", p=P)
    bproj_r = bproj.rearrange("(o p) -> p o", p=P)
    outT_r = outT.rearrange("(o p) t -> p o t", p=P)

    with tile.TileContext(nc) as tc, ExitStack() as ctx:
        const = ctx.enter_context(tc.tile_pool(name="const", bufs=1))
        qk_pool = ctx.enter_context(tc.tile_pool(name="qk", bufs=2))
        v_pool = ctx.enter_context(tc.tile_pool(name="v", bufs=2))
        attn_pool = ctx.enter_context(tc.tile_pool(name="attn", bufs=4))
        ao_pool = ctx.enter_context(tc.tile_pool(name="ao", bufs=4))
        rec_pool = ctx.enter_context(tc.tile_pool(name="rec", bufs=3))
        bc_pool = ctx.enter_context(tc.tile_pool(name="bc", bufs=3))
        out_pool = ctx.enter_context(tc.tile_pool(name="outp", bufs=3))
        dram_pool = ctx.enter_context(tc.tile_pool(name="dramp", bufs=3,
                                                   space="DRAM"))
        # PSUM: "sp" 2-bank tiles (S pair-chunks, sums, pv) + "w" filler
        sp_ps = ctx.enter_context(tc.tile_pool(name="spps", bufs=3,
                                               space="PSUM"))
        work_ps = ctx.enter_context(tc.tile_pool(name="workps", bufs=2,
                                                 space="PSUM"))

        # --- persistent tensors (per-chunk tiles => finer DMA deps) ---
        xT_ch = [const.tile([P, ntok], bf16, tag=f"xT{o}", name=f"xT{o}")
                 for o in range(FCH)]
        wqkv_ch = [const.tile([P, 3 * DIM], bf16, tag=f"wqkv{o}",
                              name=f"wqkv{o}") for o in range(FCH)]
        wproj_ch = [const.tile([P, DIM], bf16, tag=f"wproj{o}",
                               name=f"wproj{o}") for o in range(FCH)]
        # phase 1: slices the first chains touch, alternating queues
        for o in range(FCH):
            eng = nc.sync if o % 2 == 0 else nc.gpsimd
            eng.dma_start(wqkv_ch[o][:, 0:N], wqkv_r[:, o, 0:N])
            eng.dma_start(xT_ch[o][:, 0:N], xT_r[:, o, 0:N])
        # phase 2: rest of wqkv (later qk chunks + v weights)
        for o in range(FCH):
            eng = nc.sync if o % 2 == 0 else nc.gpsimd
            eng.dma_start(wqkv_ch[o][:, N:2 * DIM], wqkv_r[:, o, N:2 * DIM])
            eng.dma_start(wqkv_ch[o][:, 2 * DIM:3 * DIM],
                          wqkv_r[:, o, 2 * DIM:3 * DIM])
        # phase 3: remaining activations and proj weights
        q = (ntok - N) // 3 if n_batches > 1 else 0
        for piece in range(3 if q else 0):
            for o in range(FCH):
                eng = nc.sync if o % 2 == 0 else nc.gpsimd
                eng.dma_start(xT_ch[o][:, N + piece * q:N + (piece + 1) * q],
                              xT_r[:, o, N + piece * q:N + (piece + 1) * q])
        for o in range(FCH):
            nc.sync.dma_start(wproj_ch[o][:], wproj_r[:, o, :])
        bproj_sb = const.tile([P, FCH], f32, tag="bproj")
        nc.sync.dma_start(bproj_sb[:], bproj_r[:])
        ones_sb = const.tile([P, 1], bf16, tag="ones")
        nc.gpsimd.memset(ones_sb[:], 1.0)

        qkTs = {}   # b -> qkT tile
        v_sbs = {}  # b -> v tile
        aos = {}    # b -> attnoutT tile

        def qkv_pieces(b):
            """Yield 3-matmul filler pieces for batch b's qkv (18 chains)."""
            t0 = b * N
            qkT = qk_pool.tile([P, 2 * FCH, N], bf16, tag="qkT")
            qkTs[b] = qkT

            def qk_chain(cc):
                st = {}

                def p1():
                    st["ps"] = work_ps.tile([P, N], mybir.dt.float32,
                                            tag="w", name="fps")
                    for fo in (0, 1, 2):
                        nc.tensor.matmul(
                            st["ps"][:],
                            wqkv_ch[fo][:, cc * P:(cc + 1) * P],
                            xT_ch[fo][:, t0:t0 + N],
                            start=(fo == 0), stop=False,
                        )

                def p2():
                    for fo in (3, 4, 5):
                        nc.tensor.matmul(
                            st["ps"][:],
                            wqkv_ch[fo][:, cc * P:(cc + 1) * P],
                            xT_ch[fo][:, t0:t0 + N],
                            start=False, stop=(fo == 5),
                        )
                    if cc % 3 == 2:
                        nc.scalar.copy(qkT[:, cc, :], st["ps"][:])
                    else:
                        nc.vector.tensor_copy(qkT[:, cc, :], st["ps"][:])
                return [p1, p2]

            for cc in range(2 * FCH):
                yield from qk_chain(cc)

            v_sb = v_pool.tile([P, 3, H, HD], bf16, tag="v")
            v_sbs[b] = v_sb

            def v_chain(tch, half):
                st = {}

                def p1():
                    st["ps"] = work_ps.tile([P, N], mybir.dt.float32,
                                            tag="w", name="fps")
                    for fo in (0, 1, 2):
                        nc.tensor.matmul(
                            st["ps"][:],
                            xT_ch[fo][:, t0 + tch * P:t0 + (tch + 1) * P],
                            wqkv_ch[fo][:,
                                         2 * DIM + half * N:
                                         2 * DIM + (half + 1) * N],
                            start=(fo == 0), stop=False,
                        )

                def p2():
                    for fo in (3, 4, 5):
                        nc.tensor.matmul(
                            st["ps"][:],
                            xT_ch[fo][:, t0 + tch * P:t0 + (tch + 1) * P],
                            wqkv_ch[fo][:,
                                         2 * DIM + half * N:
                                         2 * DIM + (half + 1) * N],
                            start=False, stop=(fo == 5),
                        )
                    nc.vector.tensor_copy(
                        v_sb[:, tch, half * 6:(half + 1) * 6, :],
                        st["ps"][:].rearrange("p (h d) -> p h d", d=HD),
                    )
                return [p1, p2]

            for tch in range(3):
                for half in range(2):
                    yield from v_chain(tch, half)

        def proj_pieces(b):
            """Yield 3-matmul filler pieces for batch b's proj (6 chains)."""
            t0 = b * N
            ao = aos.pop(b)
            outstage = out_pool.tile([P, FCH, N], bf16, tag="outs")

            def proj_chain(cc):
                st = {}

                def p1():
                    st["ps"] = work_ps.tile([P, N], mybir.dt.float32,
                                            tag="w", name="fps")
                    for fo in (0, 1, 2):
                        nc.tensor.matmul(
                            st["ps"][:],
                            wproj_ch[fo][:, cc * P:(cc + 1) * P],
                            ao[:, fo, :],
                            start=(fo == 0), stop=False,
                        )

                def p2():
                    for fo in (3, 4, 5):
                        nc.tensor.matmul(
                            st["ps"][:],
                            wproj_ch[fo][:, cc * P:(cc + 1) * P],
                            ao[:, fo, :],
                            start=False, stop=(fo == 5),
                        )
                    nc.vector.tensor_scalar_add(outstage[:, cc, :],
                                                st["ps"][:],
                                                bproj_sb[:, cc:cc + 1])
                    eng = nc.sync if cc % 2 == 0 else nc.gpsimd
                    eng.dma_start(outT_r[:, cc, t0:t0 + N],
                                  outstage[:, cc, :])
                return [p1, p2]

            for cc in range(FCH):
                yield from proj_chain(cc)

        def emit_attention(b, fill):
            """Attention for batch b; fill(k) injects PE filler pieces."""
            qkT = qkTs.pop(b)
            v_sb = v_sbs[b]

            attnoutT = ao_pool.tile([P, FCH, N], bf16, tag="aoT")
            aos[b] = attnoutT

            for g in range(3):  # head groups of 4 (two pairs)
                heads = [4 * g + i for i in range(4)]
                # attnP[pair]: [P, kc, head-in-pair, N]
                attnPs = [attn_pool.tile([P, 3, 2, N], bf16, tag="attnP",
                                         name=f"attnP_{g}_{pr}")
                          for pr in range(2)]
                # S^T clump per pair: 6 row-packed mms into 3 2-bank tiles;
                # one batched exp per (pair, kc) covers both heads
                for pair in range(2):
                    j = heads[2 * pair] // 2
                    for kc in range(3):
                        sAB = sp_ps.tile([P, 2, 512], mybir.dt.float32,
                                         tag="sp", name="sAB")
                        nc.tensor.matmul(
                            sAB[:, 0, 0:N],
                            qkT[0:64, FCH + j, kc * P:(kc + 1) * P],
                            qkT[0:64, j, :], start=True, stop=True,
                            tile_position=(0, 0))
                        nc.tensor.matmul(
                            sAB[:, 1, 0:N],
                            qkT[64:128, FCH + j, kc * P:(kc + 1) * P],
                            qkT[64:128, j, :], start=True, stop=True,
                            tile_position=(64, 0))
                        nc.scalar.activation(attnPs[pair][:, kc, :, :],
                                             sAB[:, :, 0:N], Exp,
                                             scale=SCALE)
                    fill(4 if pair == 0 else 5)

                # denominators: 4 heads col-tiled; idx-inner emission so the
                # four M=1 matmuls stream concurrently
                sums = sp_ps.tile([P, NW], mybir.dt.float32, tag="sp",
                                  name="sums")
                if sim_safe:
                    nc.vector.memset(sums[:], 1.0)
                for kc in range(3):
                    for idx in range(4):
                        nc.tensor.matmul(
                            sums[32 * idx:32 * idx + 1, 0:N],
                            ones_sb[:, 0:1],
                            attnPs[idx // 2][:, kc, idx % 2, :],
                            start=(kc == 0), stop=(kc == 2),
                            tile_position=(0, 32 * idx),
                            skip_group_check=(idx > 0),
                        )
                for idx in range(4):
                    nc.tensor.matmul(
                        sums[32 * idx:32 * idx + 1, N:NW],
                        ones_sb[64:128, 0:1],
                        attnPs[idx // 2][64:128, 0, idx % 2, 64:128],
                        start=True, stop=True,
                        tile_position=(64, 32 * idx),
                        skip_group_check=(idx > 0),
                    )
                recips = rec_pool.tile([P, NW], mybir.dt.float32, tag="rec")
                nc.vector.reciprocal_approx_fast(recips[0:97, :],
                                                 sums[0:97, :])
                rec_dram = dram_pool.tile([4, NW], mybir.dt.float32, tag="rd")
                nc.gpsimd.dma_start(
                    rec_dram[:],
                    recips[:].rearrange("(a b) n -> a b n", b=32)[:, 0, :],
                )
                # broadcast: [128, pair, NW]; rows 0-63 = even (A) recips,
                # rows 64-127 = odd (B) recips
                bcast_g = bc_pool.tile([P, 2, NW], mybir.dt.float32,
                                       tag="bc")
                rec_pairs = rec_dram[:].rearrange("(p two) n -> p two n",
                                                  two=2)
                nc.gpsimd.dma_start(
                    bcast_g[0:64, :, :],
                    rec_pairs[:, 0, :].partition_broadcast(64))
                nc.gpsimd.dma_start(
                    bcast_g[64:128, :, :],
                    rec_pairs[:, 1, :].partition_broadcast(64))
                fill(2)

                # PV clump: both pairs back-to-back (needs only attn);
                # mix pair col-packed at cols 0:N, inherent at N:NW in the
                # same 2-bank tile
                pvs = []
                for pair in range(2):
                    hA, hB = heads[2 * pair], heads[2 * pair + 1]
                    attnP = attnPs[pair]
                    pv = sp_ps.tile([P, NW], mybir.dt.float32, tag="sp",
                                    name="pv")
                    pvs.append(pv)
                    for kc in range(3):
                        nc.tensor.matmul(
                            pv[0:64, 0:N], v_sb[:, kc, hA, :],
                            attnP[:, kc, 0, :],
                            start=(kc == 0), stop=(kc == 2),
                            tile_position=(0, 0),
                        )
                        nc.tensor.matmul(
                            pv[64:128, 0:N], v_sb[:, kc, hB, :],
                            attnP[:, kc, 1, :],
                            start=(kc == 0), stop=(kc == 2),
                            tile_position=(0, 64),
                            skip_group_check=True,
                        )
                    nc.tensor.matmul(
                        pv[0:64, N:NW], v_sb[64:128, 0, hA, :],
                        attnP[64:128, 0, 0, 64:128],
                        start=True, stop=True, tile_position=(64, 0),
                        skip_group_check=True,
                    )
                    nc.tensor.matmul(
                        pv[64:128, N:NW], v_sb[64:128, 0, hB, :],
                        attnP[64:128, 0, 1, 64:128],
                        start=True, stop=True, tile_position=(64, 64),
                        skip_group_check=True,
                    )
                for pair in range(2):
                    jc = heads[2 * pair] // 2
                    pv = pvs[pair]
                    nc.vector.tensor_tensor(
                        attnoutT[:, jc, 0:T_SIZE], pv[:, 0:T_SIZE],
                        bcast_g[:, pair, 0:T_SIZE], mult)
                    nc.vector.tensor_tensor(
                        attnoutT[:, jc, 2 * T_SIZE:N], pv[:, 2 * T_SIZE:N],
                        bcast_g[:, pair, 2 * T_SIZE:N], mult)
                    nc.vector.tensor_tensor(
                        attnoutT[:, jc, T_SIZE:2 * T_SIZE], pv[:, N:NW],
                        bcast_g[:, pair, N:NW], mult)
                fill(3)

        # --- emission: qkv(0) straight, then per-batch attention with
        # filler pieces from qkv(b+1) + proj(b-1); proj pieces may carry ---
        for piece in qkv_pieces(0):
            piece()

        qkv_dq = deque()
        proj_dq = deque()

        def fill(k):
            for _ in range(k):
                if qkv_dq:
                    qkv_dq.popleft()()
                elif proj_dq:
                    proj_dq.popleft()()

        for b in range(n_batches):
            if b + 1 < n_batches:
                qkv_dq.extend(qkv_pieces(b + 1))
            if b >= 1:
                proj_dq.extend(proj_pieces(b - 1))
                # cap carried proj work so pools stay bounded
                while len(proj_dq) > PROJ_CARRY_CAP:
                    proj_dq.popleft()()
            emit_attention(b, fill)
            # qkv(b+1) must complete before attention(b+1) reads qkT/v
            while qkv_dq:
                qkv_dq.popleft()()
        while proj_dq:
            proj_dq.popleft()()
        for piece in proj_pieces(n_batches - 1):
            piece()

    nc.compile()
    return nc


_CACHED_NC = None


def _get_nc():
    global _CACHED_NC
    if _CACHED_NC is None:
        _CACHED_NC = build(BPC)
    return _CACHED_NC


def kernel(x, w_qkv, w_proj, b_proj):
    from concourse.bass_utils import run_bass_kernel_spmd

    nc = _get_nc()

    wqkv_bf = np.ascontiguousarray(w_qkv.astype(BF16))
    wproj_bf = np.ascontiguousarray(w_proj.astype(BF16))
    bproj_f = np.ascontiguousarray(b_proj.astype(np.float32))

    in_maps = []
    for c in range(NCORES):
        xc = x[c * BPC:(c + 1) * BPC].reshape(BPC * N, DIM)
        xT = np.ascontiguousarray(xc.T.astype(BF16))
        in_maps.append({
            "xT": xT,
            "wqkv": wqkv_bf,
            "wproj": wproj_bf,
            "bproj": bproj_f,
        })

    res = run_bass_kernel_spmd(nc, in_maps, core_ids=list(range(NCORES)))
    outs = [
        np.ascontiguousarray(res.results[c]["outT"].T.astype(np.float32))
        .reshape(BPC, N, DIM)
        for c in range(NCORES)
    ]
    return np.concatenate(outs, axis=0)


if __name__ == "__main__":
    rng = np.random.default_rng(0)
    x = rng.standard_normal((B, N, DIM), dtype=np.float32)
    w_qkv = (rng.standard_normal((DIM, 3 * DIM), dtype=np.float32) * 0.02)
    w_proj = (rng.standard_normal((DIM, DIM), dtype=np.float32) * 0.02)
    b_proj = np.zeros((DIM,), dtype=np.float32)
    out = kernel(x, w_qkv, w_proj, b_proj)
    print("out", out.shape, out.dtype, float(np.abs(out).max()))
